# revision 18
# baseline (speedup 1.0000x reference)
"""GCNConv (message passing + linear) on 8 Trainium2 NeuronCores.

Strategy (graph/data parallel, per sharding hint):
  - Nodes sorted by (table, in-degree, A-subdegree) and dealt round-robin
    to the 8 cores (core c owns sorted-ranks {s : s % 8 == c}).
  - deg / rsqrt normalization computed on device from 0/1 slot masks.
  - Each core builds the full normalized feature table u = rsqrt(deg)*x
    (bf16) in its DRAM (two tables, int16 gather index limit), then bulk
    row-gathers per-edge messages with the Q7 dma_gather instruction and
    segment-sums them on the TensorEngine: message tile
    [128 slots, 128 feat] (stationary) x diagonal rsqrt(deg_dst) matrix
    (streaming) accumulated into PSUM — one slot per owned dst per tile.
  - Final linear via W^T matmul + bias; output is [d_out, local_dst];
    host unpermutes/transposes back to [N, d_out].

The Bass program is rebuilt per distinct edge_index (layout constants are
baked into the instruction stream); all 8 cores share one program and
differ only in their input data.
"""

import numpy as np

try:
    import ml_dtypes

    _BF16 = ml_dtypes.bfloat16
except ImportError:  # pragma: no cover
    _BF16 = None

import concourse.bacc as bacc
import concourse.bass as bass
import concourse.mybir as mybir
import concourse.tile as tile
from concourse.bass_utils import run_bass_kernel_spmd
from concourse.library_config import mlp as _mlp_lib
from concourse.masks import make_identity
from concourse.tile_rust import add_dep_helper

P = 128
N_CORES = 8
TILES_PER_CALL = 8  # gather granularity; 1024 idxs = max per dma_gather call
SPLIT_ROWS_DEFAULT = 32640  # table-A real rows (255 chunks); A size 32768


def _wrap_idx16(linear_idx):
    """[n] int -> [128, n/16] int16 in the 16-partition wrapped, 8x
    replicated layout dma_gather expects (slot i at [i%16, i//16])."""
    n = linear_idx.shape[0]
    assert n % 16 == 0
    w = linear_idx.reshape(-1, 16).T.astype(np.int16)  # [16, n/16]
    return np.tile(w, (8, 1))


# ----------------------------------------------------------------------------
# Host-side layout construction (sharding / index relabeling only — all
# floating-point math happens on device).
# ----------------------------------------------------------------------------
def _prep(x, edge_index, weight, bias, n_cores, split_rows=SPLIT_ROWS_DEFAULT):
    N, D = x.shape
    assert D == P
    src = np.asarray(edge_index[0], dtype=np.int64)
    dst = np.asarray(edge_index[1], dtype=np.int64)
    E = src.shape[0]

    deg = np.bincount(dst, minlength=N)
    count = deg + 1  # self-loop included

    CH = N // P + 1  # staged chunks; >=1 trailing zero row
    NSTAGED = CH * P

    two_tables = split_rows < N
    if two_tables:
        SPLIT = split_rows
        assert SPLIT % P == 0 and SPLIT + P <= 32768
        prelim = np.argsort(count, kind="stable")
        in_A = np.zeros(N, bool)
        in_A[prelim[:SPLIT]] = True
        # per-node count of A-source edges (self-loops handled separately)
        cntA = np.bincount(dst[in_A[src]], minlength=N)
        order = np.lexsort((cntA, count, ~in_A))
        NA = SPLIT + P  # pad rows SPLIT..NA-1 are zeroed on device
        NB = NSTAGED - SPLIT
    else:
        SPLIT = NSTAGED  # everything in table A
        in_A = np.ones(N, bool)
        cntA = deg.copy()
        order = np.argsort(count, kind="stable")
        NA = NSTAGED
        NB = 0
    cntB = deg - cntA

    rank = np.empty(N, np.int64)
    rank[order] = np.arange(N)

    count_staged = np.zeros(NSTAGED, np.int64)
    count_staged[:N] = count[order]
    cntA_staged = np.zeros(NSTAGED, np.int64)
    cntA_staged[:N] = cntA[order]
    cntB_staged = np.zeros(NSTAGED, np.int64)
    cntB_staged[:N] = cntB[order]
    x_staged = np.zeros((NSTAGED, D), np.float32)
    x_staged[:N] = np.asarray(x, dtype=np.float32)[order]

    # global 0/1 slot mask for degree counts, [128, M_total]
    Kc = np.maximum(count_staged.reshape(CH, P).max(axis=1), 1).astype(np.int64)
    offs = np.zeros(CH + 1, np.int64)
    offs[1:] = np.cumsum(Kc)
    M_total = int(offs[-1])
    mask = np.zeros((P, M_total), np.float32)
    for c in range(CH):
        K = int(Kc[c])
        mask[:, int(offs[c]) : int(offs[c]) + K] = (
            np.arange(K)[None, :] < count_staged[c * P : (c + 1) * P, None]
        )
    spans = []  # (chunk0, nchunks, K, col_off) — runs of equal K
    c0 = 0
    for c in range(1, CH + 1):
        if c == CH or Kc[c] != Kc[c0]:
            spans.append((c0, c - c0, int(Kc[c0]), int(offs[c0])))
            c0 = c

    # edges grouped by dst rank, A-sources first within each dst
    drank = rank[dst]
    src_in_B = ~in_A[src]
    eorder = np.lexsort((src_in_B, drank))
    esrc_rank = rank[src[eorder]]  # staged row of each message source
    deg_by_rank = (count[order] - 1).astype(np.int64)
    starts = np.zeros(N + 1, np.int64)
    starts[1:] = np.cumsum(deg_by_rank)

    LOCAL = (N + n_cores - 1) // n_cores
    GROUPS = (LOCAL + P - 1) // P
    LOCAL_PAD = GROUPS * P

    TgA, TgB = [], []
    for g in range(GROUPS):
        lo = n_cores * P * g
        hi = min(n_cores * P * (g + 1), N)
        if lo < N:
            TgA.append(int(cntA_staged[lo:hi].max()))
            TgB.append(int(cntB_staged[lo:hi].max()))
        else:
            TgA.append(1)
            TgB.append(0)
        if TgA[-1] + TgB[-1] == 0:
            TgA[-1] = 1
    toffsA = np.zeros(GROUPS + 1, np.int64)
    toffsA[1:] = np.cumsum(TgA)
    toffsB = np.zeros(GROUPS + 1, np.int64)
    toffsB[1:] = np.cumsum(TgB)
    T_totalA = int(toffsA[-1])
    T_totalB = int(toffsB[-1])
    T_total = T_totalA + T_totalB

    # pad slots point at guaranteed-zero rows
    PAD_A = SPLIT if two_tables else N
    PAD_B = N - SPLIT if two_tables else 0  # staged zero tail (B rows)

    x_own = np.zeros((n_cores, GROUPS * P, D), np.float32)
    for c in range(n_cores):
        k = np.arange(min((N - c + n_cores - 1) // n_cores, GROUPS * P))
        s_r = n_cores * k + c
        x_own[c][: k.shape[0]] = x_staged[s_r]

    idxA_cores = np.empty((n_cores, P, 8 * max(T_totalA, 1)), np.int16)
    idxB_cores = np.empty((n_cores, P, 8 * max(T_totalB, 1)), np.int16)
    ind_cores = np.zeros((n_cores, P, T_total), np.float32)
    prange = np.arange(P)

    for c in range(n_cores):
        linA = np.full(T_totalA * P, PAD_A, np.int64)
        linB = np.full(T_totalB * P, PAD_B, np.int64)
        for g in range(GROUPS):
            s = n_cores * (P * g + prange) + c  # global ranks of this group
            valid = s < N
            sc = np.minimum(s, N - 1)
            ca = np.where(valid, cntA_staged[sc], 0)  # A-source edges
            cb = np.where(valid, cntB_staged[sc], 0)
            st = starts[sc]

            # ---- pass A block: tiles toffsA[g] .. +TgA[g]
            TA = TgA[g]
            if TA > 0:
                colsA = np.arange(TA)[None, :]
                pickA = st[:, None] + colsA
                takeA = (colsA < ca[:, None]) & valid[:, None]
                valsA = np.where(
                    takeA, esrc_rank[np.minimum(pickA, max(E - 1, 0))], PAD_A
                )
                base = int(toffsA[g]) * P
                linA[base : base + TA * P] = valsA.T.ravel()  # tile-major
                tg0 = int(toffsA[g] + toffsB[g])
                ind_cores[c][:, tg0 : tg0 + TA] = colsA < ca[:, None]

            # ---- pass B block
            TB = TgB[g]
            if TB > 0:
                colsB = np.arange(TB)[None, :]
                pickB = st[:, None] + ca[:, None] + colsB
                takeB = (colsB < cb[:, None]) & valid[:, None]
                valsB = np.where(
                    takeB,
                    esrc_rank[np.minimum(pickB, max(E - 1, 0))] - SPLIT,
                    PAD_B,
                )
                base = int(toffsB[g]) * P
                linB[base : base + TB * P] = valsB.T.ravel()
                tg0 = int(toffsA[g] + toffsB[g]) + TA
                ind_cores[c][:, tg0 : tg0 + TB] = colsB < cb[:, None]

        assert linA.min() >= 0 and linA.max() < NA
        idxA_cores[c] = _wrap_idx16(linA) if T_totalA else 0
        if T_totalB:
            assert linB.min() >= 0 and linB.max() < NB
            idxB_cores[c] = _wrap_idx16(linB)

    wT = np.ascontiguousarray(np.asarray(weight, dtype=np.float32).T)
    bias_col = np.asarray(bias, dtype=np.float32).reshape(P, 1)

    return dict(
        N=N,
        D=D,
        E=E,
        n_cores=n_cores,
        CH=CH,
        NSTAGED=NSTAGED,
        SPLIT=SPLIT,
        NA=NA,
        NB=NB,
        two_tables=two_tables,
        M_total=M_total,
        spans=spans,
        GROUPS=GROUPS,
        LOCAL=LOCAL,
        LOCAL_PAD=LOCAL_PAD,
        TgA=TgA,
        TgB=TgB,
        toffsA=toffsA,
        toffsB=toffsB,
        T_totalA=T_totalA,
        T_totalB=T_totalB,
        T_total=T_total,
        x_staged=x_staged,
        x_own=x_own,
        mask=mask,
        ind_cores=ind_cores,
        idxA_cores=idxA_cores,
        idxB_cores=idxB_cores,
        wT=wT,
        bias_col=bias_col,
        order=order,
    )


# ----------------------------------------------------------------------------
# Device program
# ----------------------------------------------------------------------------
def _build(L, debug_taps=False, stage_limit="full"):
    CH = L["CH"]
    NSTAGED = L["NSTAGED"]
    SPLIT = L["SPLIT"]
    NA, NB = L["NA"], L["NB"]
    M_total = L["M_total"]
    GROUPS = L["GROUPS"]
    TgA, TgB = L["TgA"], L["TgB"]
    toffsA, toffsB = L["toffsA"], L["toffsB"]
    T_totalA, T_totalB = L["T_totalA"], L["T_totalB"]
    T_total = L["T_total"]
    LOCAL_PAD = L["LOCAL_PAD"]
    SPLIT_CH = SPLIT // P
    f32 = mybir.dt.float32
    bf16 = mybir.dt.bfloat16
    i16 = mybir.dt.int16
    AF = mybir.ActivationFunctionType

    nc = bacc.Bacc("TRN2", debug=False, num_devices=L["n_cores"], num_swdge_queues=4)
    x_dram = nc.dram_tensor("x_staged", [NSTAGED, P], f32, kind="ExternalInput")
    mask_dram = nc.dram_tensor("mask", [P, M_total], bf16, kind="ExternalInput")
    ind_dram = nc.dram_tensor("ind", [P, T_total], bf16, kind="ExternalInput")
    idxA_dram = nc.dram_tensor(
        "idxA", [P, 8 * max(T_totalA, 1)], i16, kind="ExternalInput"
    )
    idxB_dram = nc.dram_tensor(
        "idxB", [P, 8 * max(T_totalB, 1)], i16, kind="ExternalInput"
    )
    xown_dram = nc.dram_tensor("x_own", [LOCAL_PAD, P], f32, kind="ExternalInput")
    wT_dram = nc.dram_tensor("wT", [P, P], f32, kind="ExternalInput")
    bias_dram = nc.dram_tensor("bias_col", [P, 1], f32, kind="ExternalInput")
    out_dram = nc.dram_tensor("out", [P, LOCAL_PAD], f32, kind="ExternalOutput")

    with tile.TileContext(nc) as tc:
        with (
            tc.tile_pool(name="const", bufs=1) as cpool,
            tc.tile_pool(name="dram", bufs=1, space="DRAM") as dpool,
            tc.tile_pool(name="xw", bufs=3) as xpool,
            tc.tile_pool(name="uw", bufs=3) as upool,
            tc.tile_pool(name="msg", bufs=24) as mpool,
            tc.tile_pool(name="uself", bufs=6) as uspool,
            tc.tile_pool(name="xo", bufs=6) as xopool,
            tc.tile_pool(name="diag", bufs=6) as gpool,
            tc.tile_pool(name="agg", bufs=4) as apool,
            tc.tile_pool(name="outs", bufs=2) as opool,
            tc.tile_pool(name="ps", bufs=5, space="PSUM") as pspool,
            tc.tile_pool(name="ps2", bufs=2, space="PSUM") as ps2pool,
        ):
            if debug_taps:
                uA_dram = nc.dram_tensor("uA_dbg", [NA, P], bf16, kind="ExternalOutput")
                msg_dbg = nc.dram_tensor(
                    "msg_dbg", [P, TILES_PER_CALL, P], bf16, kind="ExternalOutput"
                )
                cntl_dbg = nc.dram_tensor(
                    "cntl_dbg", [P, GROUPS], f32, kind="ExternalOutput"
                )
            else:
                uA_dram = dpool.tile([NA, P], bf16)
            uB_dram = dpool.tile([max(NB, P), P], bf16)

            lib_inst = nc.gpsimd.load_library(_mlp_lib)

            # ---- constant loads
            mask_sb = cpool.tile([P, M_total], bf16)
            nc.sync.dma_start(out=mask_sb[:], in_=mask_dram[:])
            ind_sb = cpool.tile([P, T_total], bf16)
            nc.sync.dma_start(out=ind_sb[:], in_=ind_dram[:])
            idxA_sb = cpool.tile([P, 8 * max(T_totalA, 1)], i16)
            nc.sync.dma_start(out=idxA_sb[:], in_=idxA_dram[:])
            idxB_sb = cpool.tile([P, 8 * max(T_totalB, 1)], i16)
            nc.sync.dma_start(out=idxB_sb[:], in_=idxB_dram[:])
            wT_sb = cpool.tile([P, P], f32)
            nc.sync.dma_start(out=wT_sb[:], in_=wT_dram[:])
            bias_sb = cpool.tile([P, 1], f32)
            nc.sync.dma_start(out=bias_sb[:], in_=bias_dram[:])
            ident_sb = cpool.tile([P, P], f32)
            make_identity(nc, ident_sb[:])

            # ---- stage 0a: global counts -> dinv (per staged row)
            cnt_sb = cpool.tile([P, CH], f32)
            dinv_sb = cpool.tile([P, CH], f32)
            for (c0, nch, K, coff) in L["spans"]:
                nc.vector.reduce_sum(
                    out=cnt_sb[:, c0 : c0 + nch],
                    in_=mask_sb[:, coff : coff + nch * K].rearrange(
                        "p (n k) -> p n k", k=K
                    ),
                    axis=mybir.AxisListType.X,
                )
            nc.vector.tensor_scalar_max(cnt_sb[:], cnt_sb[:], 1.0)
            nc.scalar.sqrt(dinv_sb[:], cnt_sb[:])
            nc.vector.reciprocal(dinv_sb[:], dinv_sb[:])

            # ---- stage 0b: local (owned-dst) counts -> dinv
            cntl_sb = cpool.tile([P, GROUPS], f32)
            dinvl_sb = cpool.tile([P, GROUPS], f32)
            for g in range(GROUPS):
                t0 = int(toffsA[g] + toffsB[g])
                T = TgA[g] + TgB[g]
                nc.vector.reduce_sum(
                    out=cntl_sb[:, g : g + 1],
                    in_=ind_sb[:, t0 : t0 + T],
                    axis=mybir.AxisListType.X,
                )
            nc.vector.tensor_scalar_add(cntl_sb[:], cntl_sb[:], 1.0)  # self-loop
            if debug_taps:
                nc.sync.dma_start(out=cntl_dbg[:], in_=cntl_sb[:])
            nc.scalar.sqrt(dinvl_sb[:], cntl_sb[:])
            nc.vector.reciprocal(dinvl_sb[:], dinvl_sb[:])

            # ---- zero table-A pad rows (never written by stage 1)
            if NA > SPLIT:
                zpad = cpool.tile([P, P], bf16)
                nc.vector.memset(zpad[:], 0.0)
                nc.sync.dma_start(out=uA_dram[SPLIT:NA, :], in_=zpad[: NA - SPLIT, :])
            if NB > 0 and NSTAGED - SPLIT < NB:
                pass  # B pads are staged zero rows, written by stage 1

            # ---- stage 1: u = dinv * x (bf16) -> uA/uB
            SPAN = 4  # chunks per op
            bounds = [0, SPLIT_CH, CH] if SPLIT_CH < CH else [0, CH]
            for lo, hi in zip(bounds[:-1], bounds[1:]):
                tgt, tgt0 = (uA_dram, 0) if lo == 0 else (uB_dram, SPLIT_CH)
                for b in range(lo, hi, SPAN):
                    nch = min(SPAN, hi - b)
                    xs = xpool.tile([P, SPAN, P], f32)
                    src_ap = x_dram[b * P : (b + nch) * P, :].rearrange(
                        "(c p) f -> p c f", p=P
                    )
                    nc.sync.dma_start(out=xs[:, :nch, :], in_=src_ap)
                    us = upool.tile([P, SPAN, P], bf16)
                    nc.vector.tensor_tensor(
                        out=us[:, :nch, :],
                        in0=xs[:, :nch, :],
                        in1=dinv_sb[:, b : b + nch].broadcast_to([P, nch, P]),
                        op=mybir.AluOpType.mult,
                    )
                    r0 = (b - tgt0) * P
                    dst_ap = tgt[r0 : r0 + nch * P, :].rearrange(
                        "(c p) f -> p c f", p=P
                    )
                    nc.sync.dma_start(out=dst_ap, in_=us[:, :nch, :])

            # ---- stage 2: gather + segment-sum (PE) + linear + bias
            if stage_limit == "s1":
                dummy = cpool.tile([P, LOCAL_PAD], f32)
                nc.vector.memset(dummy[:], 0.0)
                nc.sync.dma_start(out=out_dram[:], in_=dummy[:])
            msg_tiles = {}
            qrr = [0]

            def ensure_call(pass_key, k):
                key = (pass_key, k)
                if key in msg_tiles:
                    return
                T_tot = T_totalA if pass_key == "A" else T_totalB
                u_src = uA_dram if pass_key == "A" else uB_dram
                idx_sb = idxA_sb if pass_key == "A" else idxB_sb
                t0 = k * TILES_PER_CALL
                cnt = min(TILES_PER_CALL, T_tot - t0)
                m = mpool.tile([P, TILES_PER_CALL, P], bf16)
                g_inst = nc.gpsimd.dma_gather(
                    m[:, :cnt, :],
                    u_src[:, :],
                    idx_sb[:, 8 * t0 : 8 * (t0 + cnt)],
                    cnt * P,
                    cnt * P,
                    P,
                    queue_num=qrr[0] % 4,
                )
                qrr[0] += 1
                add_dep_helper(g_inst.ins, lib_inst.ins, reason="ucode lib before gather")
                msg_tiles[key] = m
                if debug_taps and pass_key == "A" and k == 0:
                    nc.sync.dma_start(out=msg_dbg[:], in_=m[:])

            if stage_limit == "gather":
                probe = cpool.tile([P, LOCAL_PAD], f32)
                nc.vector.memset(probe[:], 0.0)
                n_callsA = (T_totalA + TILES_PER_CALL - 1) // TILES_PER_CALL
                n_callsB = (T_totalB + TILES_PER_CALL - 1) // TILES_PER_CALL
                pi = 0
                for pk, ncalls in (("A", n_callsA), ("B", n_callsB)):
                    for k in range(ncalls):
                        ensure_call(pk, k)
                        nc.vector.tensor_copy(
                            out=probe[:, (pi % (LOCAL_PAD // P)) * P :][:, :P],
                            in_=msg_tiles[(pk, k)][:, 0, :],
                        )
                        pi += 1
                nc.sync.dma_start(out=out_dram[:], in_=probe[:])

            out_t = None
            ostart = 0
            for g in range(GROUPS if stage_limit == "full" else 0):
                diag = gpool.tile([P, P], bf16)
                nc.scalar.mul(diag[:], ident_sb[:], dinvl_sb[:, g : g + 1])
                xo = xopool.tile([P, P], f32)
                nc.sync.dma_start(out=xo[:], in_=xown_dram[g * P : (g + 1) * P, :])
                uself = uspool.tile([P, P], bf16)
                nc.scalar.mul(uself[:], xo[:], dinvl_sb[:, g : g + 1])
                psum = pspool.tile([P, P], f32)
                T = TgA[g] + TgB[g] + 1
                j = 0
                for pass_key, Tp, toffs in (
                    ("A", TgA[g], toffsA),
                    ("B", TgB[g], toffsB),
                ):
                    for jj in range(Tp):
                        t = int(toffs[g]) + jj
                        k, kk = divmod(t, TILES_PER_CALL)
                        ensure_call(pass_key, k)
                        nc.tensor.matmul(
                            out=psum[:],
                            lhsT=msg_tiles[(pass_key, k)][:, kk, :],
                            rhs=diag[:],
                            start=(j == 0),
                            stop=False,
                        )
                        j += 1
                nc.tensor.matmul(
                    out=psum[:],
                    lhsT=uself[:],
                    rhs=diag[:],
                    start=(j == 0),
                    stop=True,
                )
                agg = apool.tile([P, P], f32)
                nc.vector.tensor_copy(out=agg[:], in_=psum[:])
                psum2 = ps2pool.tile([P, P], f32)
                nc.tensor.matmul(
                    out=psum2[:], lhsT=wT_sb[:], rhs=agg[:], start=True, stop=True
                )
                ob = g % 4
                if ob == 0:
                    out_t = opool.tile([P, 4 * P], f32)
                    ostart = g
                nc.scalar.activation(
                    out_t[:, ob * P : (ob + 1) * P],
                    psum2[:],
                    AF.Identity,
                    bias=bias_sb[:, 0:1],
                )
                if ob == 3 or g == GROUPS - 1:
                    w = (g - ostart + 1) * P
                    nc.sync.dma_start(
                        out=out_dram[:, ostart * P : ostart * P + w],
                        in_=out_t[:, :w],
                    )

    nc.compile()
    return nc


def _in_maps(L):
    maps = []
    for c in range(L["n_cores"]):
        maps.append(
            {
                "x_staged": L["x_staged"],
                "x_own": L["x_own"][c],
                "mask": L["mask"].astype(_BF16),
                "ind": L["ind_cores"][c].astype(_BF16),
                "idxA": L["idxA_cores"][c],
                "idxB": L["idxB_cores"][c],
                "wT": L["wT"],
                "bias_col": L["bias_col"],
            }
        )
    return maps


def _assemble(L, outs):
    N = L["N"]
    n_cores = L["n_cores"]
    LOCAL = L["LOCAL"]
    order = L["order"]
    res = np.empty((N, P), np.float32)
    for c in range(n_cores):
        oc = np.asarray(outs[c]["out"])  # [128, LOCAL_PAD]
        k = np.arange(LOCAL)
        s = n_cores * k + c
        m = s < N
        res[order[s[m]]] = oc[:, :LOCAL][:, m].T
    return res


_CACHE = {}
LAST_EXEC_NS = None


def kernel(x, edge_index, weight, bias, *, trace=False, n_cores=N_CORES):
    global LAST_EXEC_NS
    x = np.asarray(x, dtype=np.float32)
    edge_index = np.asarray(edge_index)
    weight = np.asarray(weight, dtype=np.float32)
    bias = np.asarray(bias, dtype=np.float32)

    key = hash(edge_index.tobytes()) ^ hash((x.shape, n_cores))
    if key in _CACHE:
        L, nc = _CACHE[key]
        xs = np.zeros((L["NSTAGED"], P), np.float32)
        xs[: L["N"]] = x[L["order"]]
        L["x_staged"] = xs
        L["wT"] = np.ascontiguousarray(weight.T)
        L["bias_col"] = bias.reshape(P, 1)
    else:
        L = _prep(x, edge_index, weight, bias, n_cores)
        nc = _build(L)
        _CACHE.clear()
        _CACHE[key] = (L, nc)

    res = run_bass_kernel_spmd(
        nc, _in_maps(L), core_ids=list(range(n_cores)), trace=trace
    )
    LAST_EXEC_NS = res.exec_time_ns
    return _assemble(L, res.results)


# revision 19
# speedup vs baseline: 1.7066x; 1.7066x over previous
"""GCNConv (message passing + linear) on 8 Trainium2 NeuronCores.

Strategy (graph/data parallel, per sharding hint):
  - Nodes sorted by (table, in-degree, A-subdegree) and dealt round-robin
    to the 8 cores (core c owns sorted-ranks {s : s % 8 == c}).
  - deg / rsqrt normalization computed on device from 0/1 slot masks.
  - Each core builds the full normalized feature table u = rsqrt(deg)*x
    (bf16) in its DRAM (two tables, int16 gather index limit), then bulk
    row-gathers per-edge messages with the Q7 dma_gather instruction and
    segment-sums them on the TensorEngine: message tile
    [128 slots, 128 feat] (stationary) x diagonal rsqrt(deg_dst) matrix
    (streaming) accumulated into PSUM — one slot per owned dst per tile.
  - Final linear via W^T matmul + bias; output is [d_out, local_dst];
    host unpermutes/transposes back to [N, d_out].

The Bass program is rebuilt per distinct edge_index (layout constants are
baked into the instruction stream); all 8 cores share one program and
differ only in their input data.
"""

import numpy as np

try:
    import ml_dtypes

    _BF16 = ml_dtypes.bfloat16
except ImportError:  # pragma: no cover
    _BF16 = None

import concourse.bacc as bacc
import concourse.bass as bass
import concourse.mybir as mybir
import concourse.tile as tile
from concourse.bass_utils import run_bass_kernel_spmd
from concourse.library_config import mlp as _mlp_lib
from concourse.masks import make_identity
from concourse.tile_rust import add_dep_helper

P = 128
N_CORES = 8
TILES_PER_CALL = 8  # gather granularity; 1024 idxs = max per dma_gather call
SPLIT_ROWS_DEFAULT = 32640  # table-A real rows (255 chunks); A size 32768


def _wrap_idx16(linear_idx):
    """[n] int -> [128, n/16] int16 in the 16-partition wrapped, 8x
    replicated layout dma_gather expects (slot i at [i%16, i//16])."""
    n = linear_idx.shape[0]
    assert n % 16 == 0
    w = linear_idx.reshape(-1, 16).T.astype(np.int16)  # [16, n/16]
    return np.tile(w, (8, 1))


# ----------------------------------------------------------------------------
# Host-side layout construction (sharding / index relabeling only — all
# floating-point math happens on device).
# ----------------------------------------------------------------------------
def _prep(x, edge_index, weight, bias, n_cores, split_rows=SPLIT_ROWS_DEFAULT):
    N, D = x.shape
    assert D == P
    src = np.asarray(edge_index[0], dtype=np.int64)
    dst = np.asarray(edge_index[1], dtype=np.int64)
    E = src.shape[0]

    deg = np.bincount(dst, minlength=N)
    count = deg + 1  # self-loop included

    CH = N // P + 1  # staged chunks; >=1 trailing zero row
    NSTAGED = CH * P

    two_tables = split_rows < N
    if two_tables:
        SPLIT = split_rows
        assert SPLIT % P == 0 and SPLIT + P <= 32768
        prelim = np.argsort(count, kind="stable")
        in_A = np.zeros(N, bool)
        in_A[prelim[:SPLIT]] = True
        # per-node count of A-source edges (self-loops handled separately)
        cntA = np.bincount(dst[in_A[src]], minlength=N)
        cntB_pre = deg - cntA
        snake = np.where(cntA % 2 == 0, cntB_pre, (1 << 20) - cntB_pre)
        order = np.lexsort((snake, cntA, ~in_A))
        NA = SPLIT + P  # pad rows SPLIT..NA-1 are zeroed on device
        NB = NSTAGED - SPLIT
    else:
        SPLIT = NSTAGED  # everything in table A
        in_A = np.ones(N, bool)
        cntA = deg.copy()
        order = np.argsort(count, kind="stable")
        NA = NSTAGED
        NB = 0
    cntB = deg - cntA

    rank = np.empty(N, np.int64)
    rank[order] = np.arange(N)

    count_staged = np.zeros(NSTAGED, np.int64)
    count_staged[:N] = count[order]
    cntA_staged = np.zeros(NSTAGED, np.int64)
    cntA_staged[:N] = cntA[order]
    cntB_staged = np.zeros(NSTAGED, np.int64)
    cntB_staged[:N] = cntB[order]
    x_staged = np.zeros((NSTAGED, D), np.float32)
    x_staged[:N] = np.asarray(x, dtype=np.float32)[order]

    # global 0/1 slot mask for degree counts, [128, M_total]
    Kc = np.maximum(count_staged.reshape(CH, P).max(axis=1), 1).astype(np.int64)
    offs = np.zeros(CH + 1, np.int64)
    offs[1:] = np.cumsum(Kc)
    M_total = int(offs[-1])
    mask = np.zeros((P, M_total), np.float32)
    for c in range(CH):
        K = int(Kc[c])
        mask[:, int(offs[c]) : int(offs[c]) + K] = (
            np.arange(K)[None, :] < count_staged[c * P : (c + 1) * P, None]
        )
    spans = []  # (chunk0, nchunks, K, col_off) — runs of equal K
    c0 = 0
    for c in range(1, CH + 1):
        if c == CH or Kc[c] != Kc[c0]:
            spans.append((c0, c - c0, int(Kc[c0]), int(offs[c0])))
            c0 = c

    # edges grouped by dst rank, A-sources first within each dst
    drank = rank[dst]
    src_in_B = ~in_A[src]
    eorder = np.lexsort((src_in_B, drank))
    esrc_rank = rank[src[eorder]]  # staged row of each message source
    deg_by_rank = (count[order] - 1).astype(np.int64)
    starts = np.zeros(N + 1, np.int64)
    starts[1:] = np.cumsum(deg_by_rank)

    LOCAL = (N + n_cores - 1) // n_cores
    GROUPS = (LOCAL + P - 1) // P
    LOCAL_PAD = GROUPS * P

    TgA, TgB = [], []
    for g in range(GROUPS):
        lo = n_cores * P * g
        hi = min(n_cores * P * (g + 1), N)
        if lo < N:
            TgA.append(int(cntA_staged[lo:hi].max()))
            TgB.append(int(cntB_staged[lo:hi].max()))
        else:
            TgA.append(1)
            TgB.append(0)
        if TgA[-1] + TgB[-1] == 0:
            TgA[-1] = 1
    toffsA = np.zeros(GROUPS + 1, np.int64)
    toffsA[1:] = np.cumsum(TgA)
    toffsB = np.zeros(GROUPS + 1, np.int64)
    toffsB[1:] = np.cumsum(TgB)
    T_totalA = int(toffsA[-1])
    T_totalB = int(toffsB[-1])
    T_total = T_totalA + T_totalB

    # pad slots point at guaranteed-zero rows
    PAD_A = SPLIT if two_tables else N
    PAD_B = N - SPLIT if two_tables else 0  # staged zero tail (B rows)

    x_own = np.zeros((n_cores, GROUPS * P, D), np.float32)
    for c in range(n_cores):
        k = np.arange(min((N - c + n_cores - 1) // n_cores, GROUPS * P))
        s_r = n_cores * k + c
        x_own[c][: k.shape[0]] = x_staged[s_r]

    idxA_cores = np.empty((n_cores, P, 8 * max(T_totalA, 1)), np.int16)
    idxB_cores = np.empty((n_cores, P, 8 * max(T_totalB, 1)), np.int16)
    ind_cores = np.zeros((n_cores, P, T_total), np.float32)
    prange = np.arange(P)

    for c in range(n_cores):
        linA = np.full(T_totalA * P, PAD_A, np.int64)
        linB = np.full(T_totalB * P, PAD_B, np.int64)
        for g in range(GROUPS):
            s = n_cores * (P * g + prange) + c  # global ranks of this group
            valid = s < N
            sc = np.minimum(s, N - 1)
            ca = np.where(valid, cntA_staged[sc], 0)  # A-source edges
            cb = np.where(valid, cntB_staged[sc], 0)
            st = starts[sc]

            # ---- pass A block: tiles toffsA[g] .. +TgA[g]
            TA = TgA[g]
            if TA > 0:
                colsA = np.arange(TA)[None, :]
                pickA = st[:, None] + colsA
                takeA = (colsA < ca[:, None]) & valid[:, None]
                valsA = np.where(
                    takeA, esrc_rank[np.minimum(pickA, max(E - 1, 0))], PAD_A
                )
                base = int(toffsA[g]) * P
                linA[base : base + TA * P] = valsA.T.ravel()  # tile-major
                tg0 = int(toffsA[g] + toffsB[g])
                ind_cores[c][:, tg0 : tg0 + TA] = colsA < ca[:, None]

            # ---- pass B block
            TB = TgB[g]
            if TB > 0:
                colsB = np.arange(TB)[None, :]
                pickB = st[:, None] + ca[:, None] + colsB
                takeB = (colsB < cb[:, None]) & valid[:, None]
                valsB = np.where(
                    takeB,
                    esrc_rank[np.minimum(pickB, max(E - 1, 0))] - SPLIT,
                    PAD_B,
                )
                base = int(toffsB[g]) * P
                linB[base : base + TB * P] = valsB.T.ravel()
                tg0 = int(toffsA[g] + toffsB[g]) + TA
                ind_cores[c][:, tg0 : tg0 + TB] = colsB < cb[:, None]

        assert linA.min() >= 0 and linA.max() < NA
        idxA_cores[c] = _wrap_idx16(linA) if T_totalA else 0
        if T_totalB:
            assert linB.min() >= 0 and linB.max() < NB
            idxB_cores[c] = _wrap_idx16(linB)

    wT = np.ascontiguousarray(np.asarray(weight, dtype=np.float32).T)
    bias_col = np.asarray(bias, dtype=np.float32).reshape(P, 1)

    return dict(
        N=N,
        D=D,
        E=E,
        n_cores=n_cores,
        CH=CH,
        NSTAGED=NSTAGED,
        SPLIT=SPLIT,
        NA=NA,
        NB=NB,
        two_tables=two_tables,
        M_total=M_total,
        spans=spans,
        GROUPS=GROUPS,
        LOCAL=LOCAL,
        LOCAL_PAD=LOCAL_PAD,
        TgA=TgA,
        TgB=TgB,
        toffsA=toffsA,
        toffsB=toffsB,
        T_totalA=T_totalA,
        T_totalB=T_totalB,
        T_total=T_total,
        x_staged=x_staged,
        x_own=x_own,
        mask=mask,
        ind_cores=ind_cores,
        idxA_cores=idxA_cores,
        idxB_cores=idxB_cores,
        wT=wT,
        bias_col=bias_col,
        order=order,
    )


# ----------------------------------------------------------------------------
# Device program
# ----------------------------------------------------------------------------
def _build(L, debug_taps=False, stage_limit="full"):
    CH = L["CH"]
    NSTAGED = L["NSTAGED"]
    SPLIT = L["SPLIT"]
    NA, NB = L["NA"], L["NB"]
    M_total = L["M_total"]
    GROUPS = L["GROUPS"]
    TgA, TgB = L["TgA"], L["TgB"]
    toffsA, toffsB = L["toffsA"], L["toffsB"]
    T_totalA, T_totalB = L["T_totalA"], L["T_totalB"]
    T_total = L["T_total"]
    LOCAL_PAD = L["LOCAL_PAD"]
    SPLIT_CH = SPLIT // P
    f32 = mybir.dt.float32
    bf16 = mybir.dt.bfloat16
    i16 = mybir.dt.int16
    AF = mybir.ActivationFunctionType

    nc = bacc.Bacc("TRN2", debug=False, num_devices=L["n_cores"], num_swdge_queues=4)
    x_dram = nc.dram_tensor("x_staged", [NSTAGED, P], f32, kind="ExternalInput")
    mask_dram = nc.dram_tensor("mask", [P, M_total], bf16, kind="ExternalInput")
    ind_dram = nc.dram_tensor("ind", [P, T_total], bf16, kind="ExternalInput")
    idxA_dram = nc.dram_tensor(
        "idxA", [P, 8 * max(T_totalA, 1)], i16, kind="ExternalInput"
    )
    idxB_dram = nc.dram_tensor(
        "idxB", [P, 8 * max(T_totalB, 1)], i16, kind="ExternalInput"
    )
    xown_dram = nc.dram_tensor("x_own", [LOCAL_PAD, P], f32, kind="ExternalInput")
    wT_dram = nc.dram_tensor("wT", [P, P], f32, kind="ExternalInput")
    bias_dram = nc.dram_tensor("bias_col", [P, 1], f32, kind="ExternalInput")
    out_dram = nc.dram_tensor("out", [P, LOCAL_PAD], f32, kind="ExternalOutput")

    with tile.TileContext(nc) as tc:
        with (
            tc.tile_pool(name="const", bufs=1) as cpool,
            tc.tile_pool(name="dram", bufs=1, space="DRAM") as dpool,
            tc.tile_pool(name="xw", bufs=3) as xpool,
            tc.tile_pool(name="uw", bufs=3) as upool,
            tc.tile_pool(name="msg", bufs=24) as mpool,
            tc.tile_pool(name="uself", bufs=6) as uspool,
            tc.tile_pool(name="xo", bufs=6) as xopool,
            tc.tile_pool(name="diag", bufs=6) as gpool,
            tc.tile_pool(name="agg", bufs=4) as apool,
            tc.tile_pool(name="outs", bufs=2) as opool,
            tc.tile_pool(name="ps", bufs=5, space="PSUM") as pspool,
            tc.tile_pool(name="ps2", bufs=2, space="PSUM") as ps2pool,
        ):
            if debug_taps:
                uA_dram = nc.dram_tensor("uA_dbg", [NA, P], bf16, kind="ExternalOutput")
                msg_dbg = nc.dram_tensor(
                    "msg_dbg", [P, TILES_PER_CALL, P], bf16, kind="ExternalOutput"
                )
                cntl_dbg = nc.dram_tensor(
                    "cntl_dbg", [P, GROUPS], f32, kind="ExternalOutput"
                )
            else:
                uA_dram = dpool.tile([NA, P], bf16)
            uB_dram = dpool.tile([max(NB, P), P], bf16)

            lib_inst = nc.gpsimd.load_library(_mlp_lib)

            # ---- constant loads
            mask_sb = cpool.tile([P, M_total], bf16)
            nc.sync.dma_start(out=mask_sb[:], in_=mask_dram[:])
            ind_sb = cpool.tile([P, T_total], bf16)
            nc.sync.dma_start(out=ind_sb[:], in_=ind_dram[:])
            idxA_sb = cpool.tile([P, 8 * max(T_totalA, 1)], i16)
            nc.sync.dma_start(out=idxA_sb[:], in_=idxA_dram[:])
            idxB_sb = cpool.tile([P, 8 * max(T_totalB, 1)], i16)
            nc.sync.dma_start(out=idxB_sb[:], in_=idxB_dram[:])
            wT_sb = cpool.tile([P, P], f32)
            nc.sync.dma_start(out=wT_sb[:], in_=wT_dram[:])
            bias_sb = cpool.tile([P, 1], f32)
            nc.sync.dma_start(out=bias_sb[:], in_=bias_dram[:])
            ident_sb = cpool.tile([P, P], f32)
            make_identity(nc, ident_sb[:])

            # ---- stage 0a: global counts -> dinv (per staged row)
            cnt_sb = cpool.tile([P, CH], f32)
            dinv_sb = cpool.tile([P, CH], f32)
            for (c0, nch, K, coff) in L["spans"]:
                nc.vector.reduce_sum(
                    out=cnt_sb[:, c0 : c0 + nch],
                    in_=mask_sb[:, coff : coff + nch * K].rearrange(
                        "p (n k) -> p n k", k=K
                    ),
                    axis=mybir.AxisListType.X,
                )
            nc.vector.tensor_scalar_max(cnt_sb[:], cnt_sb[:], 1.0)
            nc.scalar.sqrt(dinv_sb[:], cnt_sb[:])
            nc.vector.reciprocal(dinv_sb[:], dinv_sb[:])

            # ---- stage 0b: local (owned-dst) counts -> dinv
            cntl_sb = cpool.tile([P, GROUPS], f32)
            dinvl_sb = cpool.tile([P, GROUPS], f32)
            for g in range(GROUPS):
                t0 = int(toffsA[g] + toffsB[g])
                T = TgA[g] + TgB[g]
                nc.vector.reduce_sum(
                    out=cntl_sb[:, g : g + 1],
                    in_=ind_sb[:, t0 : t0 + T],
                    axis=mybir.AxisListType.X,
                )
            nc.vector.tensor_scalar_add(cntl_sb[:], cntl_sb[:], 1.0)  # self-loop
            if debug_taps:
                nc.sync.dma_start(out=cntl_dbg[:], in_=cntl_sb[:])
            nc.scalar.sqrt(dinvl_sb[:], cntl_sb[:])
            nc.vector.reciprocal(dinvl_sb[:], dinvl_sb[:])

            # ---- zero table-A pad rows (never written by stage 1)
            if NA > SPLIT:
                zpad = cpool.tile([P, P], bf16)
                nc.vector.memset(zpad[:], 0.0)
                nc.sync.dma_start(out=uA_dram[SPLIT:NA, :], in_=zpad[: NA - SPLIT, :])
            if NB > 0 and NSTAGED - SPLIT < NB:
                pass  # B pads are staged zero rows, written by stage 1

            # ---- stage 1: u = dinv * x (bf16) -> uA/uB
            SPAN = 8  # chunks per op
            bounds = [0, SPLIT_CH, CH] if SPLIT_CH < CH else [0, CH]
            for lo, hi in zip(bounds[:-1], bounds[1:]):
                tgt, tgt0 = (uA_dram, 0) if lo == 0 else (uB_dram, SPLIT_CH)
                for b in range(lo, hi, SPAN):
                    nch = min(SPAN, hi - b)
                    xs = xpool.tile([P, SPAN, P], f32)
                    src_ap = x_dram[b * P : (b + nch) * P, :].rearrange(
                        "(c p) f -> p c f", p=P
                    )
                    nc.sync.dma_start(out=xs[:, :nch, :], in_=src_ap)
                    us = upool.tile([P, SPAN, P], bf16)
                    nc.vector.tensor_tensor(
                        out=us[:, :nch, :],
                        in0=xs[:, :nch, :],
                        in1=dinv_sb[:, b : b + nch].broadcast_to([P, nch, P]),
                        op=mybir.AluOpType.mult,
                    )
                    r0 = (b - tgt0) * P
                    dst_ap = tgt[r0 : r0 + nch * P, :].rearrange(
                        "(c p) f -> p c f", p=P
                    )
                    nc.scalar.dma_start(out=dst_ap, in_=us[:, :nch, :])

            # ---- stage 2: gather + segment-sum (PE) + linear + bias
            if stage_limit == "s1":
                dummy = cpool.tile([P, LOCAL_PAD], f32)
                nc.vector.memset(dummy[:], 0.0)
                nc.sync.dma_start(out=out_dram[:], in_=dummy[:])
            msg_tiles = {}
            qrr = [0]

            def ensure_call(pass_key, k):
                key = (pass_key, k)
                if key in msg_tiles:
                    return
                T_tot = T_totalA if pass_key == "A" else T_totalB
                u_src = uA_dram if pass_key == "A" else uB_dram
                idx_sb = idxA_sb if pass_key == "A" else idxB_sb
                t0 = k * TILES_PER_CALL
                cnt = min(TILES_PER_CALL, T_tot - t0)
                m = mpool.tile([P, TILES_PER_CALL, P], bf16)
                g_inst = nc.gpsimd.dma_gather(
                    m[:, :cnt, :],
                    u_src[:, :],
                    idx_sb[:, 8 * t0 : 8 * (t0 + cnt)],
                    cnt * P,
                    cnt * P,
                    P,
                    queue_num=qrr[0] % 4,
                )
                qrr[0] += 1
                add_dep_helper(g_inst.ins, lib_inst.ins, reason="ucode lib before gather")
                msg_tiles[key] = m
                if debug_taps and pass_key == "A" and k == 0:
                    nc.sync.dma_start(out=msg_dbg[:], in_=m[:])

            if stage_limit == "gather":
                probe = cpool.tile([P, LOCAL_PAD], f32)
                nc.vector.memset(probe[:], 0.0)
                n_callsA = (T_totalA + TILES_PER_CALL - 1) // TILES_PER_CALL
                n_callsB = (T_totalB + TILES_PER_CALL - 1) // TILES_PER_CALL
                pi = 0
                for pk, ncalls in (("A", n_callsA), ("B", n_callsB)):
                    for k in range(ncalls):
                        ensure_call(pk, k)
                        nc.vector.tensor_copy(
                            out=probe[:, (pi % (LOCAL_PAD // P)) * P :][:, :P],
                            in_=msg_tiles[(pk, k)][:, 0, :],
                        )
                        pi += 1
                nc.sync.dma_start(out=out_dram[:], in_=probe[:])

            out_t = None
            ostart = 0
            for g in range(GROUPS if stage_limit == "full" else 0):
                diag = gpool.tile([P, P], bf16)
                nc.scalar.mul(diag[:], ident_sb[:], dinvl_sb[:, g : g + 1])
                xo = xopool.tile([P, P], f32)
                nc.sync.dma_start(out=xo[:], in_=xown_dram[g * P : (g + 1) * P, :])
                uself = uspool.tile([P, P], bf16)
                nc.scalar.mul(uself[:], xo[:], dinvl_sb[:, g : g + 1])
                psum = pspool.tile([P, P], f32)
                T = TgA[g] + TgB[g] + 1
                j = 0
                for pass_key, Tp, toffs in (
                    ("A", TgA[g], toffsA),
                    ("B", TgB[g], toffsB),
                ):
                    for jj in range(Tp):
                        t = int(toffs[g]) + jj
                        k, kk = divmod(t, TILES_PER_CALL)
                        ensure_call(pass_key, k)
                        nc.tensor.matmul(
                            out=psum[:],
                            lhsT=msg_tiles[(pass_key, k)][:, kk, :],
                            rhs=diag[:],
                            start=(j == 0),
                            stop=False,
                        )
                        j += 1
                nc.tensor.matmul(
                    out=psum[:],
                    lhsT=uself[:],
                    rhs=diag[:],
                    start=(j == 0),
                    stop=True,
                )
                agg = apool.tile([P, P], f32)
                nc.vector.tensor_copy(out=agg[:], in_=psum[:])
                psum2 = ps2pool.tile([P, P], f32)
                nc.tensor.matmul(
                    out=psum2[:], lhsT=wT_sb[:], rhs=agg[:], start=True, stop=True
                )
                ob = g % 4
                if ob == 0:
                    out_t = opool.tile([P, 4 * P], f32)
                    ostart = g
                nc.scalar.activation(
                    out_t[:, ob * P : (ob + 1) * P],
                    psum2[:],
                    AF.Identity,
                    bias=bias_sb[:, 0:1],
                )
                if ob == 3 or g == GROUPS - 1:
                    w = (g - ostart + 1) * P
                    nc.sync.dma_start(
                        out=out_dram[:, ostart * P : ostart * P + w],
                        in_=out_t[:, :w],
                    )

    nc.compile()
    return nc


def _in_maps(L):
    maps = []
    for c in range(L["n_cores"]):
        maps.append(
            {
                "x_staged": L["x_staged"],
                "x_own": L["x_own"][c],
                "mask": L["mask"].astype(_BF16),
                "ind": L["ind_cores"][c].astype(_BF16),
                "idxA": L["idxA_cores"][c],
                "idxB": L["idxB_cores"][c],
                "wT": L["wT"],
                "bias_col": L["bias_col"],
            }
        )
    return maps


def _assemble(L, outs):
    N = L["N"]
    n_cores = L["n_cores"]
    LOCAL = L["LOCAL"]
    order = L["order"]
    res = np.empty((N, P), np.float32)
    for c in range(n_cores):
        oc = np.asarray(outs[c]["out"])  # [128, LOCAL_PAD]
        k = np.arange(LOCAL)
        s = n_cores * k + c
        m = s < N
        res[order[s[m]]] = oc[:, :LOCAL][:, m].T
    return res


_CACHE = {}
LAST_EXEC_NS = None


def kernel(x, edge_index, weight, bias, *, trace=False, n_cores=N_CORES):
    global LAST_EXEC_NS
    x = np.asarray(x, dtype=np.float32)
    edge_index = np.asarray(edge_index)
    weight = np.asarray(weight, dtype=np.float32)
    bias = np.asarray(bias, dtype=np.float32)

    key = hash(edge_index.tobytes()) ^ hash((x.shape, n_cores))
    if key in _CACHE:
        L, nc = _CACHE[key]
        xs = np.zeros((L["NSTAGED"], P), np.float32)
        xs[: L["N"]] = x[L["order"]]
        L["x_staged"] = xs
        L["wT"] = np.ascontiguousarray(weight.T)
        L["bias_col"] = bias.reshape(P, 1)
    else:
        L = _prep(x, edge_index, weight, bias, n_cores)
        nc = _build(L)
        _CACHE.clear()
        _CACHE[key] = (L, nc)

    res = run_bass_kernel_spmd(
        nc, _in_maps(L), core_ids=list(range(n_cores)), trace=trace
    )
    LAST_EXEC_NS = res.exec_time_ns
    return _assemble(L, res.results)
